# revision 3
# baseline (speedup 1.0000x reference)
"""Time-varying FIR (AllZeroDigitalFilter) on 8 TRN2 NeuronCores.

Math: y[b, k*P+i] = w0[i]*A[b,k,i] + w1[i]*B[b,k,i]
  A[b,k,:] = conv of x-window of frame k with fixed filter h[b,k,:]
  B[b,k,:] = same window with h[b,k+1,:] (last frame replicated)
  w1[i] = i/P, w0 = 1-w1   ->   y = A + w1*(B-A)

Layout: partition = frame (125/tile), free = extended 129-sample window.
Per tap j one fused DVE op: acc = (XE[:, 49-j : 129-j] * h[:, j]) + acc.
Sharding: pure data parallel, 2 sequences per core.
"""

import sys

for p in ("/opt/trn_rl_repo", "/root/.axon_site/_ro/trn_rl_repo"):
    if p not in sys.path:
        sys.path.append(p)

import numpy as np
import concourse.bass as bass
import concourse.mybir as mybir
from concourse.ap import AP
from concourse.bass_utils import run_bass_kernel_spmd

B, T = 16, 80000
P, D = 80, 50  # frame period, taps (order M=49)
N = T // P  # 1000 frames
W = P + D - 1  # 129 extended window
NCORES = 8
S = B // NCORES  # 2 sequences per core
F = 125  # frames per tile
NT = N // F  # 8 tiles per sequence
TP = T + D - 1  # padded x length

FP32 = mybir.dt.float32

_nc_cache = {}


def build_nc():
    if "nc" in _nc_cache:
        return _nc_cache["nc"]
    nc = bass.Bass()
    xp_ext = nc.declare_dram_parameter("xp", [S, TP], FP32, isOutput=False)
    ha_ext = nc.declare_dram_parameter("ha", [S, N, D], FP32, isOutput=False)
    hb_ext = nc.declare_dram_parameter("hb", [S, N, D], FP32, isOutput=False)
    w1_ext = nc.declare_dram_parameter("w1", [128, P], FP32, isOutput=False)
    out_ext = nc.declare_dram_parameter("out", [S, T], FP32, isOutput=True)

    NTILES = S * NT  # 16 tiles total, tile t -> seq s = t // NT, chunk i = t % NT

    with (
        nc.sbuf_tensor([F, W], FP32) as xe0,
        nc.sbuf_tensor([F, W], FP32) as xe1,
        nc.sbuf_tensor([F, D], FP32) as ha0,
        nc.sbuf_tensor([F, D], FP32) as ha1,
        nc.sbuf_tensor([F, D], FP32) as hb0,
        nc.sbuf_tensor([F, D], FP32) as hb1,
        nc.sbuf_tensor([F, P], FP32) as accA0,
        nc.sbuf_tensor([F, P], FP32) as accA1,
        nc.sbuf_tensor([F, P], FP32) as accB0,
        nc.sbuf_tensor([F, P], FP32) as accB1,
        nc.sbuf_tensor([F, P], FP32) as tmp,
        nc.sbuf_tensor([F, P], FP32) as y0,
        nc.sbuf_tensor([F, P], FP32) as y1,
        nc.sbuf_tensor([128, P], FP32) as w1t,
        nc.semaphore("ramp_sem") as ramp_sem,
        nc.semaphore("dma_e") as dma_e,
        nc.semaphore("dma_o") as dma_o,
        nc.semaphore("dve_sem") as dve_sem,
        nc.semaphore("out_e") as out_e,
        nc.semaphore("out_o") as out_o,
        nc.Block() as block,
    ):
        xe = [xe0, xe1]
        ha = [ha0, ha1]
        hb = [hb0, hb1]
        yb = [y0, y1]
        dma_s = [dma_e, dma_o]
        out_s = [out_e, out_o]

        # NOTE on sync design: cumulative thresholds on a shared DMA semaphore
        # are UNSOUND when >1 DMA is in flight (per-SDMA-engine completion skew
        # lets later tiles' increments satisfy an earlier tile's threshold).
        # Parity semaphores make each threshold equal to the max possible
        # increment count at wait time, so a fired wait implies full completion.

        @block.sync
        def _(sync):
            sync.dma_start(w1t[:], w1_ext[:]).then_inc(ramp_sem, 16)
            for t in range(NTILES):
                s, ci = t // NT, t % NT
                b = t % 2
                k0 = ci * F
                if t >= 2:
                    # WAR: tile t-2 finished all reads of buffer set b
                    sync.wait_ge(dve_sem, t - 1)
                src = AP(
                    tensor=xp_ext[:].tensor,
                    offset=s * TP + k0 * P,
                    ap=[[P, F], [1, W]],
                )
                sync.dma_start(xe[b][:], src).then_inc(dma_s[b], 16)
                sync.dma_start(ha[b][:], ha_ext[s, k0 : k0 + F, :]).then_inc(
                    dma_s[b], 16
                )
                sync.dma_start(hb[b][:], hb_ext[s, k0 : k0 + F, :]).then_inc(
                    dma_s[b], 16
                )
                if t >= 1:
                    sync.wait_ge(dve_sem, t)
                    tp = t - 1
                    sp, cip = tp // NT, tp % NT
                    dst = AP(
                        tensor=out_ext[:].tensor,
                        offset=sp * T + cip * F * P,
                        ap=[[P, F], [1, P]],
                    )
                    sync.dma_start(dst, yb[tp % 2][:]).then_inc(out_s[tp % 2], 16)
            sync.wait_ge(dve_sem, NTILES)
            tp = NTILES - 1
            sp, cip = tp // NT, tp % NT
            dst = AP(
                tensor=out_ext[:].tensor,
                offset=sp * T + cip * F * P,
                ap=[[P, F], [1, P]],
            )
            sync.dma_start(dst, yb[tp % 2][:]).then_inc(out_s[tp % 2], 16)
            sync.wait_ge(out_s[tp % 2], 16 * (tp // 2 + 1))
            sync.wait_ge(out_s[1 - tp % 2], 16 * ((tp - 1) // 2 + 1))

        @block.vector
        def _(vector):
            def conv_pass(xbuf, hbuf, acc_a, acc_b):
                # returns buffer holding the final 50-tap accumulation
                vector.tensor_scalar_mul(
                    acc_a[:], xbuf[:, D - 1 : D - 1 + P], hbuf[:, 0:1]
                )
                accs = [acc_a, acc_b]
                cur = 0
                for j in range(1, D):
                    nxt = 1 - cur
                    vector.scalar_tensor_tensor(
                        out=accs[nxt][:],
                        in0=xbuf[:, D - 1 - j : D - 1 - j + P],
                        scalar=hbuf[:, j : j + 1],
                        in1=accs[cur][:],
                        op0=mybir.AluOpType.mult,
                        op1=mybir.AluOpType.add,
                    )
                    cur = nxt
                return accs[cur]

            vector.wait_ge(ramp_sem, 16)
            for t in range(NTILES):
                b = t % 2
                vector.wait_ge(dma_s[b], 48 * (t // 2 + 1))
                fa = conv_pass(xe[b], ha[b], accA0, accA1)
                fb = conv_pass(xe[b], hb[b], accB0, accB1)
                if t >= 2:
                    # WAR: out-DMA of tile t-2 done with y buffer b
                    vector.wait_ge(out_s[b], 16 * (t // 2))
                # y = A + w1 * (B - A)
                vector.tensor_tensor(
                    out=tmp[:], in0=fb[:], in1=fa[:], op=mybir.AluOpType.subtract
                )
                vector.tensor_tensor(
                    out=tmp[:], in0=tmp[:], in1=w1t[0:F, :], op=mybir.AluOpType.mult
                )
                vector.tensor_tensor(
                    out=yb[b][:], in0=tmp[:], in1=fa[:], op=mybir.AluOpType.add
                ).then_inc(dve_sem, 1)

    _nc_cache["nc"] = nc
    return nc


def _prep_core_inputs(x, h):
    """Full inputs -> list of 8 per-core input dicts."""
    x = np.ascontiguousarray(x, dtype=np.float32)
    h = np.ascontiguousarray(h, dtype=np.float32)
    xp = np.zeros((B, TP), np.float32)
    xp[:, D - 1 :] = x
    # B filter: h[k+1] with last frame replicated (matches diffsptk interp)
    hb = np.concatenate([h[:, 1:, :], h[:, -1:, :]], axis=1)
    w1 = np.broadcast_to((np.arange(P, dtype=np.float32) / P)[None, :], (128, P))
    w1 = np.ascontiguousarray(w1)
    in_maps = []
    for c in range(NCORES):
        sl = slice(c * S, (c + 1) * S)
        in_maps.append(
            {
                "xp": xp[sl],
                "ha": h[sl],
                "hb": hb[sl],
                "w1": w1,
            }
        )
    return in_maps


def kernel(x, h, **kw):
    nc = build_nc()
    in_maps = _prep_core_inputs(x, h)
    res = run_bass_kernel_spmd(nc, in_maps, core_ids=list(range(NCORES)), **kw)
    out = np.concatenate([res.results[c]["out"] for c in range(NCORES)], axis=0)
    return np.ascontiguousarray(out, dtype=np.float32)


def kernel_traced(x, h, **kw):
    """Like kernel() but returns (out, BassKernelResults) with profile info."""
    nc = build_nc()
    in_maps = _prep_core_inputs(x, h)
    res = run_bass_kernel_spmd(
        nc, in_maps, core_ids=list(range(NCORES)), trace=True, **kw
    )
    out = np.concatenate([res.results[c]["out"] for c in range(NCORES)], axis=0)
    return np.ascontiguousarray(out, dtype=np.float32), res


# revision 10
# speedup vs baseline: 1.3829x; 1.3829x over previous
"""Time-varying FIR (AllZeroDigitalFilter) on 8 TRN2 NeuronCores.

V2: fp16 "C-decomposition".
  C_k[i'] = sum_j h_pad[k,j] * x[(k-1)P + i' - j],  i' in [0,160)
  (filter of frame k applied across frames k-1 and k)
  y[kP+i] = w0[i]*C_k[80+i] + w1[i]*C_{k+1}[i]
One fused DVE op (scalar_tensor_tensor, FD=160) per tap -> 50 ops/tile
instead of 100 FD=80 ops. fp16 gives the DVE 2x perf mode; precision
validated at ~1.1e-3 relative error. The cross-partition (+1) combine is
done with a partition-shifted SBUF->SBUF DMA + one tensor add.
Sharding: pure data parallel, 2 sequences per core.
"""

import sys

for p in ("/opt/trn_rl_repo", "/root/.axon_site/_ro/trn_rl_repo"):
    if p not in sys.path:
        sys.path.append(p)

import numpy as np
import concourse.bass as bass
import concourse.mybir as mybir
from concourse.ap import AP
from concourse.bass_utils import run_bass_kernel_spmd

B, T = 16, 80000
P, D = 80, 50  # frame period, taps
N = T // P  # 1000 frames
W2 = 2 * P + D - 1  # 209: extended window for the 160-wide C rows
NCORES = 8
S = B // NCORES  # sequences per core
FO = 125  # output frames per tile
FT = FO + 1  # C-rows per tile (tiles overlap by 1 row)
NTSEQ = N // FO  # 8 tiles per sequence
PAD = D - 1 + P  # front pad so C_k window starts at k*P: 129
TPC = N * P + W2 + 2  # padded x length (+2 slack for the odd-offset copy)

F16 = mybir.dt.float16
FP32 = mybir.dt.float32

_nc_cache = {}


def build_nc():
    if "nc" in _nc_cache:
        return _nc_cache["nc"]
    nc = bass.Bass()
    xp_ext = nc.declare_dram_parameter("xp", [S, TPC], F16, isOutput=False)
    hc_ext = nc.declare_dram_parameter("hc", [S, N + 1, D], FP32, isOutput=False)
    rr_ext = nc.declare_dram_parameter("rr", [128, 2 * P], F16, isOutput=False)
    out_ext = nc.declare_dram_parameter("out", [S, T], FP32, isOutput=True)
    ydram = nc.dram_tensor("ydram", [S, T], F16)

    NTILES = S * NTSEQ  # tile t -> seq s = t // NTSEQ, chunk ci = t % NTSEQ

    from contextlib import ExitStack

    with ExitStack() as _ctx:
        ec = _ctx.enter_context
        xa0 = ec(nc.sbuf_tensor([FT, W2], F16))
        xa1 = ec(nc.sbuf_tensor([FT, W2], F16))
        xb0 = ec(nc.sbuf_tensor([FT, W2], F16))
        xb1 = ec(nc.sbuf_tensor([FT, W2], F16))
        hh0 = ec(nc.sbuf_tensor([FT, D], FP32))
        hh1 = ec(nc.sbuf_tensor([FT, D], FP32))
        acc0 = ec(nc.sbuf_tensor([FT, 2 * P], F16))
        acc1 = ec(nc.sbuf_tensor([FT, 2 * P], F16))
        vt = ec(nc.sbuf_tensor([FT, 2 * P], F16))
        vs = ec(nc.sbuf_tensor([FO, P], F16))
        y0 = ec(nc.sbuf_tensor([FO, P], F16))
        y1 = ec(nc.sbuf_tensor([FO, P], F16))
        rrt = ec(nc.sbuf_tensor([128, 2 * P], F16))
        ramp_sem = ec(nc.semaphore("ramp_sem"))
        dma_e = ec(nc.semaphore("dma_e"))
        dma_o = ec(nc.semaphore("dma_o"))
        v_sem = ec(nc.semaphore("v_sem"))
        vs_sem = ec(nc.semaphore("vs_sem"))
        ya_sem = ec(nc.semaphore("ya_sem"))
        out_e = ec(nc.semaphore("out_e"))
        out_o = ec(nc.semaphore("out_o"))
        cast_sem = ec(nc.semaphore("cast_sem"))
        block = ec(nc.Block())
        xa = [xa0, xa1]
        xb = [xb0, xb1]
        hh = [hh0, hh1]
        yt = [y0, y1]
        dma_s = [dma_e, dma_o]
        out_s = [out_e, out_o]

        def ydst(t):
            s, ci = t // NTSEQ, t % NTSEQ
            return AP(
                tensor=ydram[:].tensor,
                offset=s * T + ci * FO * P,
                ap=[[P, FO], [1, P]],
            )

        @block.sync
        def _(sync):
            sync.dma_start(rrt[:], rr_ext[:]).then_inc(ramp_sem, 16)
            for t in range(NTILES):
                s, ci = t // NTSEQ, t % NTSEQ
                b = t % 2
                k0 = ci * FO
                if t >= 2:
                    sync.wait_ge(v_sem, t - 1)  # WAR: tile t-2 read its inputs
                src_a = AP(
                    tensor=xp_ext[:].tensor,
                    offset=s * TPC + k0 * P,
                    ap=[[P, FT], [1, W2]],
                )
                src_b = AP(
                    tensor=xp_ext[:].tensor,
                    offset=s * TPC + k0 * P + 1,
                    ap=[[P, FT], [1, W2]],
                )
                sync.dma_start(xa[b][:], src_a).then_inc(dma_s[b], 16)
                sync.dma_start(xb[b][:], src_b).then_inc(dma_s[b], 16)
                sync.dma_start(hh[b][:], hc_ext[s, k0 : k0 + FT, :]).then_inc(
                    dma_s[b], 16
                )
                if t >= 1:
                    # partition-shift copy of V rows 1..FT for tile t-1
                    sync.wait_ge(v_sem, t)
                    sync.dma_start(vs[:], vt[1:FT, 0:P]).then_inc(vs_sem, 16)
                if t >= 2:
                    # store y of tile t-2 (ya available early; avoids blocking
                    # the next tile's input DMAs behind tile t-1's compute)
                    sync.wait_ge(ya_sem, t - 1)
                    sync.dma_start(ydst(t - 2), yt[(t - 2) % 2][:]).then_inc(
                        out_s[(t - 2) % 2], 16
                    )
            # tail: last tile's shift + remaining stores
            tl = NTILES - 1
            sync.wait_ge(v_sem, NTILES)
            sync.dma_start(vs[:], vt[1:FT, 0:P]).then_inc(vs_sem, 16)
            sync.wait_ge(ya_sem, NTILES - 1)
            sync.dma_start(ydst(tl - 1), yt[(tl - 1) % 2][:]).then_inc(
                out_s[(tl - 1) % 2], 16
            )
            sync.wait_ge(ya_sem, NTILES)
            sync.dma_start(ydst(tl), yt[tl % 2][:]).then_inc(out_s[tl % 2], 16)
            sync.wait_ge(out_s[tl % 2], 16 * (tl // 2 + 1))
            sync.wait_ge(out_s[1 - tl % 2], 16 * ((tl - 1) // 2 + 1))
            sync.sem_inc(cast_sem, 1)
            # after the gpsimd cast pass completes, kernel may end
            sync.wait_ge(cast_sem, 17)

        @block.vector
        def _(vector):
            def conv(t):
                b = t % 2
                accs = [acc0, acc1]
                vector.wait_ge(dma_s[b], 48 * (t // 2 + 1))
                # j=0: offset D-1 (odd) -> use xb (the +1-shifted copy) at D-2
                vector.tensor_scalar_mul(
                    acc0[:], xb[b][:, D - 2 : D - 2 + 2 * P], hh[b][:, 0:1]
                )
                cur = 0
                for j in range(1, D):
                    nxt = 1 - cur
                    off = D - 1 - j
                    if off % 2 == 0:
                        src = xa[b][:, off : off + 2 * P]
                    else:
                        src = xb[b][:, off - 1 : off - 1 + 2 * P]
                    vector.scalar_tensor_tensor(
                        out=accs[nxt][:],
                        in0=src,
                        scalar=hh[b][:, j : j + 1],
                        in1=accs[cur][:],
                        op0=mybir.AluOpType.mult,
                        op1=mybir.AluOpType.add,
                    )
                    cur = nxt
                return accs[cur]

            vector.wait_ge(ramp_sem, 16)
            for t in range(NTILES):
                fin = conv(t)
                if t >= 1:
                    # combine tile t-1: y = V[0:FO, 80:160] + Vs
                    vector.wait_ge(vs_sem, 16 * t)
                    if t - 1 >= 2:
                        vector.wait_ge(out_s[(t - 1) % 2], 16 * ((t - 1) // 2))
                    vector.tensor_tensor(
                        out=yt[(t - 1) % 2][:],
                        in0=vt[0:FO, P : 2 * P],
                        in1=vs[:],
                        op=mybir.AluOpType.add,
                    ).then_inc(ya_sem, 1)
                # V_t = C_t * rr
                vector.tensor_tensor(
                    out=vt[:], in0=fin[:], in1=rrt[0:FT, :], op=mybir.AluOpType.mult
                ).then_inc(v_sem, 1)
            # tail combine for last tile
            tl = NTILES - 1
            vector.wait_ge(vs_sem, 16 * NTILES)
            vector.wait_ge(out_s[tl % 2], 16 * (tl // 2))
            vector.tensor_tensor(
                out=yt[tl % 2][:],
                in0=vt[0:FO, P : 2 * P],
                in1=vs[:],
                op=mybir.AluOpType.add,
            ).then_inc(ya_sem, 1)

        @block.gpsimd
        def _(gpsimd):
            # final cast pass fp16 -> fp32 (SWDGE dtype-cast DMA)
            gpsimd.wait_ge(cast_sem, 1)
            gpsimd.dma_start(out_ext[:], ydram[:]).then_inc(cast_sem, 16)

    _nc_cache["nc"] = nc
    return nc


def _prep_core_inputs(x, h):
    x = np.ascontiguousarray(x, dtype=np.float32)
    h = np.ascontiguousarray(h, dtype=np.float32)
    xp = np.zeros((B, TPC), np.float16)
    xp[:, PAD : PAD + T] = x.astype(np.float16)
    hpad = np.ascontiguousarray(np.concatenate([h, h[:, -1:, :]], axis=1))  # (B,N+1,D) f32
    w1 = (np.arange(P, dtype=np.float32) / P).astype(np.float16)
    w0 = (1.0 - np.arange(P, dtype=np.float32) / P).astype(np.float16)
    rr = np.broadcast_to(
        np.concatenate([w1, w0])[None, :], (128, 2 * P)
    )
    rr = np.ascontiguousarray(rr)
    in_maps = []
    for c in range(NCORES):
        sl = slice(c * S, (c + 1) * S)
        in_maps.append({"xp": xp[sl], "hc": hpad[sl], "rr": rr})
    return in_maps


def kernel(x, h, **kw):
    nc = build_nc()
    in_maps = _prep_core_inputs(x, h)
    res = run_bass_kernel_spmd(nc, in_maps, core_ids=list(range(NCORES)), **kw)
    out = np.concatenate([res.results[c]["out"] for c in range(NCORES)], axis=0)
    return np.ascontiguousarray(out, dtype=np.float32)


def kernel_traced(x, h, **kw):
    nc = build_nc()
    in_maps = _prep_core_inputs(x, h)
    res = run_bass_kernel_spmd(
        nc, in_maps, core_ids=list(range(NCORES)), trace=True, **kw
    )
    out = np.concatenate([res.results[c]["out"] for c in range(NCORES)], axis=0)
    return np.ascontiguousarray(out, dtype=np.float32), res
